# revision 10
# baseline (speedup 1.0000x reference)
# Trainium2 Bass kernel for nn_Attention_80779744903426
#
# Reference computation (b=4, n=2048, c=1024, h=16, d=64):
#   qkv = x @ w_qkv ; split to q,k,v per head
#   attn = softmax(q k^T / sqrt(c)) ; out = (attn v) concat ; y = out @ w_proj + b_proj
#
# Sharding (8 cores): data-parallel over batch (4) x tensor-parallel over
# head-groups (2 groups of 8 heads, Megatron-style). Each core computes a
# partial y for its batch from its 8 heads; host sums the two partials per
# batch and adds b_proj.
#
# Per-core program (all matmuls via PE in float32r, N>=256 for full rate):
#   A) qk^T = wqk^T @ x^T   -> staged to DRAM  [1024, 2048] (Q^T rows 0:512, K^T rows 512:1024)
#      V    = x @ wv        -> SBUF, stored per (k-tile, head) with a ones column appended
#   B) per head pair (row-tiled K=64 matmuls) and q-chunk:
#      S^T[k,q] = K^T_h(stationary) x Q^T_h(moving); exp via ACT (scale folded);
#      O'[65,q] = [V_h | 1]^T @ P~^T accumulated over k tiles (fused softmax denominator);
#      normalize rows 0:64 by row 64 via DVE (reciprocal + partition-broadcast DMA + mult)
#   C) y = O^T(stationary) @ wp(moving), accumulated over 4 o-tiles, DMA to DRAM.

import numpy as np

DIM = 1024
N = 2048
B = 4
NH = 16
HD = 64
SCALE = 1.0 / DIM**0.5

HPC = 8            # heads per core
PAIRS = HPC // 2   # head pairs (row-tiled together)
CT = 8             # contraction tiles over c=1024
NT = 16            # n tiles of 128
ACH = 512          # phase-A n-chunk
QCH = 256          # phase-B q-chunk
NQC = N // QCH     # 8 q-chunks
KT = 16            # k tiles of 128 in attention

USE_F32R = True
S_BATCHES = [(0, 6), (6, 6), (12, 4)]  # k-tile batches per (head, q-chunk) for ACT exp

_CACHE = {}


def _build_nc():
    import concourse.bass as bass
    from concourse import bacc, mybir, tile

    f32 = mybir.dt.float32
    f32r = mybir.dt.float32r if USE_F32R else f32
    EXP = mybir.ActivationFunctionType.Exp

    def mm(out, lhsT, rhs, **kw):
        nc.tensor.matmul(out, lhsT.bitcast(f32r), rhs.bitcast(f32r), **kw)

    nc = bacc.Bacc("TRN2", target_bir_lowering=False, debug=False)

    xT_d = nc.dram_tensor("xT", [DIM, N], f32, kind="ExternalInput").ap()
    wqk_d = nc.dram_tensor("wqk", [DIM, 1024], f32, kind="ExternalInput").ap()
    wv_d = nc.dram_tensor("wv", [DIM, 512], f32, kind="ExternalInput").ap()
    wp_d = nc.dram_tensor("wp", [512, DIM], f32, kind="ExternalInput").ap()
    y_d = nc.dram_tensor("y", [N, DIM], f32, kind="ExternalOutput").ap()

    with tile.TileContext(nc) as tc:
        with (
            tc.tile_pool(name="p16", bufs=3) as p16,      # 16KB slots: xT chunks <-> P~ tiles
            tc.tile_pool(name="wqk", bufs=1) as wqkp,     # 32KB
            tc.tile_pool(name="wv", bufs=1) as wvp,       # 16KB
            tc.tile_pool(name="wp", bufs=1) as wpp,       # 16KB
            tc.tile_pool(name="v", bufs=1) as vp,         # 33.3KB
            tc.tile_pool(name="ot", bufs=1) as otp,       # 32KB
            tc.tile_pool(name="kt", bufs=2) as ktp,       # 8KB x2
            tc.tile_pool(name="qt", bufs=2) as qtp,       # 1KB x2
            tc.tile_pool(name="misc", bufs=2) as miscp,
            tc.tile_pool(name="ps", bufs=2, space="PSUM") as psp,
            tc.tile_pool(name="dram", bufs=1, space="DRAM") as dp,
        ):
            qkT_d = dp.tile([DIM, N], f32, name="qkT_stage")
            # ---- static tiles ----
            wqk_sb = wqkp.tile([128, CT, 1024], f32)
            for ct in range(CT):
                nc.sync.dma_start(wqk_sb[:, ct, :].bitcast(f32r),
                                  wqk_d[128 * ct : 128 * (ct + 1), :].bitcast(f32r))
            wv_sb = wvp.tile([128, CT, 512], f32)
            for ct in range(CT):
                nc.sync.dma_start(wv_sb[:, ct, :].bitcast(f32r),
                                  wv_d[128 * ct : 128 * (ct + 1), :].bitcast(f32r))
            wp_sb = wpp.tile([128, 4, 1024], f32)
            for ot in range(4):
                nc.sync.dma_start(wp_sb[:, ot, :].bitcast(f32r),
                                  wp_d[128 * ot : 128 * (ot + 1), :].bitcast(f32r))

            v_sb = vp.tile([128, NT, HPC, HD + 1], f32)  # [k-part, k-tile, head, d | ones]
            ones_sb = miscp.tile([128, HPC], f32, tag="ones", bufs=1)
            nc.vector.memset(ones_sb, 1.0)
            for nt in range(NT):
                nc.vector.tensor_copy(v_sb[:, nt, :, HD].bitcast(f32r), ones_sb)

            ot_sb = otp.tile([128, PAIRS, N], f32)  # O^T rows: pair p = rows 128p..128p+127

            xT_r = xT_d.rearrange("(t p) n -> p t n", p=128)

            # ---- phase A: qkv projections ----
            # mt 0..3 = Q^T pairs, mt 4..7 = K^T pairs; emit K/Q for pair 0 first
            mt_order = [4, 0, 5, 1, 6, 2, 7, 3]
            for ach in range(N // ACH):
                xt = p16.tile([128, CT, ACH], f32, tag="big16")
                nc.sync.dma_start(xt.bitcast(f32r),
                                  xT_r[:, :, ACH * ach : ACH * (ach + 1)].bitcast(f32r))
                # V = x @ wv : out [n-tile, 512]
                for sub in range(ACH // 128):
                    nt = (ACH // 128) * ach + sub
                    vps = psp.tile([128, 512], f32, tag="sbatch")
                    for ct in range(CT):
                        mm(vps, xt[:, ct, 128 * sub : 128 * (sub + 1)], wv_sb[:, ct, :],
                           start=(ct == 0), stop=(ct == CT - 1))
                    nc.vector.tensor_copy(
                        v_sb[:, nt, :, 0:HD].bitcast(f32r),
                        vps.rearrange("p (h d) -> p h d", h=HPC),
                    )
                # qk^T = wqk^T @ x^T : out [m-tile, n-chunk] -> DRAM stage
                for mt in mt_order:
                    qps = psp.tile([128, 512], f32, tag="sbatch")
                    for ct in range(CT):
                        mm(qps, wqk_sb[:, ct, 128 * mt : 128 * (mt + 1)], xt[:, ct, :],
                           start=(ct == 0), stop=(ct == CT - 1))
                    stg = miscp.tile([128, 512], f32, tag="stg", bufs=3)
                    nc.vector.tensor_copy(stg, qps)
                    nc.sync.dma_start(
                        qkT_d[128 * mt : 128 * (mt + 1), ACH * ach : ACH * (ach + 1)], stg
                    )

            # ---- phase B: attention per head-pair ----
            for p in range(PAIRS):
                kt_sb = ktp.tile([128, N], f32)  # K^T rows for both heads of the pair
                nc.sync.dma_start(kt_sb.bitcast(f32r),
                                  qkT_d[512 + 128 * p : 512 + 128 * (p + 1), :].bitcast(f32r))
                for qc in range(NQC):
                    qt_sb = qtp.tile([128, QCH], f32)
                    nc.sync.dma_start(
                        qt_sb.bitcast(f32r),
                        qkT_d[128 * p : 128 * (p + 1), QCH * qc : QCH * (qc + 1)].bitcast(f32r),
                    )
                    ptiles = [
                        p16.tile([128, KT, QCH], f32, tag="big16", name=f"pt{hh}")
                        for hh in range(2)
                    ]
                    # S^T + exp: row-tiled pairs (head A rows 0:64, head B rows 64:128)
                    for b0, bn in S_BATCHES:
                        for hh in range(2):
                            sl = slice(64 * hh, 64 * (hh + 1))
                            sps = psp.tile([128, 6, QCH], f32, tag="sbatch")
                            for i in range(bn):
                                k = b0 + i
                                mm(sps[:, i, :], kt_sb[sl, 128 * k : 128 * (k + 1)],
                                   qt_sb[sl, :], start=True, stop=True)
                            nc.scalar.activation(
                                out=ptiles[hh][:, b0 : b0 + bn, :].bitcast(f32r),
                                in_=sps[:, 0:bn, :],
                                func=EXP,
                                scale=float(SCALE),
                            )
                    # O' = [V | 1]^T @ P~^T, then normalize by fused row sums
                    for hh in range(2):
                        h = 2 * p + hh
                        ops = psp.tile([HD + 1, QCH], f32, tag="acc")
                        for k in range(KT):
                            mm(ops, v_sb[:, k, h, :], ptiles[hh][:, k, :],
                               start=(k == 0), stop=(k == KT - 1))
                        rcp = miscp.tile([1, QCH], f32, tag="rcp")
                        nc.vector.reciprocal(rcp, ops[HD : HD + 1, :])
                        # partition-broadcast via DRAM bounce (stride-0 partition
                        # APs are only legal on DRAM sources)
                        rcp_d = dp.tile([1, QCH], f32, tag="rcpd", bufs=4, name="rcpd")
                        nc.sync.dma_start(rcp_d, rcp)
                        bc = miscp.tile([64, QCH], f32, tag="bc")
                        rap = rcp_d[:]
                        nc.sync.dma_start(
                            bc,
                            bass.AP(tensor=rap.tensor, offset=rap.offset,
                                    ap=[[0, 64]] + list(rap.ap[1:])),
                        )
                        nc.vector.tensor_mul(
                            ot_sb[64 * hh : 64 * (hh + 1), p, QCH * qc : QCH * (qc + 1)].bitcast(f32r),
                            ops[0:HD, :],
                            bc,
                        )

            # ---- phase C: y = O @ wp ----
            for nt in range(NT):
                for yc in range(2):
                    yps = psp.tile([128, 512], f32, tag="acc")
                    for ot in range(4):
                        mm(yps, ot_sb[:, ot, 128 * nt : 128 * (nt + 1)],
                           wp_sb[:, ot, 512 * yc : 512 * (yc + 1)],
                           start=(ot == 0), stop=(ot == 3))
                    stg = miscp.tile([128, 512], f32, tag="stg", bufs=3)
                    nc.vector.tensor_copy(stg, yps)
                    nc.sync.dma_start(
                        y_d[128 * nt : 128 * (nt + 1), 512 * yc : 512 * (yc + 1)], stg
                    )

    nc.compile()
    return nc


def get_nc():
    if "nc" not in _CACHE:
        _CACHE["nc"] = _build_nc()
    return _CACHE["nc"]


def make_in_maps(x, w_qkv, w_proj):
    in_maps = []
    for c in range(8):
        b, g = c // 2, c % 2
        in_maps.append({
            "xT": np.ascontiguousarray(x[b].T, dtype=np.float32),
            "wqk": np.ascontiguousarray(
                np.concatenate(
                    [w_qkv[:, 512 * g : 512 * (g + 1)],
                     w_qkv[:, 1024 + 512 * g : 1024 + 512 * (g + 1)]], axis=1
                ), dtype=np.float32),
            "wv": np.ascontiguousarray(
                w_qkv[:, 2048 + 512 * g : 2048 + 512 * (g + 1)], dtype=np.float32),
            "wp": np.ascontiguousarray(
                w_proj[512 * g : 512 * (g + 1), :], dtype=np.float32),
        })
    return in_maps


def kernel(x, w_qkv, w_proj, b_proj):
    from concourse.bass_utils import run_bass_kernel_spmd

    x = np.asarray(x, dtype=np.float32)
    w_qkv = np.asarray(w_qkv, dtype=np.float32)
    w_proj = np.asarray(w_proj, dtype=np.float32)
    b_proj = np.asarray(b_proj, dtype=np.float32)

    nc = get_nc()
    in_maps = make_in_maps(x, w_qkv, w_proj)
    res = run_bass_kernel_spmd(nc, in_maps, list(range(8))).results

    out = np.zeros((B, N, DIM), dtype=np.float32)
    for c in range(8):
        out[c // 2] += res[c]["y"]
    return out + b_proj


# revision 11
# speedup vs baseline: 1.0232x; 1.0232x over previous
# Trainium2 Bass kernel for nn_Attention_80779744903426
#
# Reference computation (b=4, n=2048, c=1024, h=16, d=64):
#   qkv = x @ w_qkv ; split to q,k,v per head
#   attn = softmax(q k^T / sqrt(c)) ; out = (attn v) concat ; y = out @ w_proj + b_proj
#
# Sharding (8 cores): data-parallel over batch (4) x tensor-parallel over
# head-groups (2 groups of 8 heads, Megatron-style). Each core computes a
# partial y for its batch from its 8 heads; host sums the two partials per
# batch and adds b_proj.
#
# Per-core program:
#   A) (fp32r) qk^T = wqk^T @ x^T -> staged to DRAM as bf16 [1024, 2048]
#      (Q^T rows 0:512, K^T rows 512:1024);  V = x @ wv -> SBUF bf16, stored
#      per (k-tile, head) with a ones column appended
#   B) (bf16) per head pair (row-tiled K=64 matmuls) and q-chunk of 512:
#      S^T[k,q] = K^T_h(stationary) x Q^T_h(moving); exp via ACT over 4-bank
#      PSUM batches (scale folded), written bf16;
#      O'[65,q] = [V_h | 1]^T @ P~^T accumulated over 16 k-tiles (fused
#      softmax denominator); normalize rows 0:64 by row 64 via DVE
#      (reciprocal + partition-broadcast DMA via DRAM bounce + multiply)
#   C) (fp32r) y = O^T(stationary) @ wp(moving), accumulated over 4 o-tiles.

import numpy as np

DIM = 1024
N = 2048
B = 4
NH = 16
HD = 64
SCALE = 1.0 / DIM**0.5

HPC = 8            # heads per core
PAIRS = HPC // 2   # head pairs (row-tiled together)
CT = 8             # contraction tiles over c=1024
NT = 16            # n tiles of 128
ACH = 512          # phase-A n-chunk
QCH = 512          # phase-B q-chunk
NQC = N // QCH     # 4 q-chunks
KT = 16            # k tiles of 128 in attention

S_BATCHES = [(0, 4), (4, 4), (8, 4), (12, 4)]  # k-tile batches per (head, q-chunk)

_CACHE = {}


def _build_nc():
    import concourse.bass as bass
    from concourse import bacc, mybir, tile

    f32 = mybir.dt.float32
    f32r = mybir.dt.float32r
    bf16 = mybir.dt.bfloat16
    EXP = mybir.ActivationFunctionType.Exp

    def mmr(out, lhsT, rhs, **kw):  # fp32r matmul on fp32 storage
        nc.tensor.matmul(out, lhsT.bitcast(f32r), rhs.bitcast(f32r), **kw)

    nc = bacc.Bacc("TRN2", target_bir_lowering=False, debug=False)

    xT_d = nc.dram_tensor("xT", [DIM, N], f32, kind="ExternalInput").ap()
    wqk_d = nc.dram_tensor("wqk", [DIM, 1024], f32, kind="ExternalInput").ap()
    wv_d = nc.dram_tensor("wv", [DIM, 512], f32, kind="ExternalInput").ap()
    wp_d = nc.dram_tensor("wp", [512, DIM], f32, kind="ExternalInput").ap()
    y_d = nc.dram_tensor("y", [N, DIM], f32, kind="ExternalOutput").ap()

    with tile.TileContext(nc) as tc:
        with (
            tc.tile_pool(name="p16", bufs=3) as p16,      # 16KB slots: xT chunks <-> P~ tiles
            tc.tile_pool(name="wqk", bufs=1) as wqkp,     # 32KB
            tc.tile_pool(name="wv", bufs=1) as wvp,       # 16KB
            tc.tile_pool(name="wp", bufs=1) as wpp,       # 16KB
            tc.tile_pool(name="v", bufs=1) as vp,         # 16.6KB bf16
            tc.tile_pool(name="ot", bufs=1) as otp,       # 32KB f32
            tc.tile_pool(name="kt", bufs=2) as ktp,       # 4KB x2 bf16
            tc.tile_pool(name="qt", bufs=2) as qtp,       # 1KB x2 bf16
            tc.tile_pool(name="misc", bufs=2) as miscp,
            tc.tile_pool(name="ps", bufs=1, space="PSUM") as psp,
            tc.tile_pool(name="dram", bufs=1, space="DRAM") as dp,
        ):
            qkT_d = dp.tile([DIM, N], bf16, name="qkT_stage")
            # ---- static tiles ----
            wqk_sb = wqkp.tile([128, CT, 1024], f32)
            for ct in range(CT):
                nc.sync.dma_start(wqk_sb[:, ct, :].bitcast(f32r),
                                  wqk_d[128 * ct : 128 * (ct + 1), :].bitcast(f32r))
            wv_sb = wvp.tile([128, CT, 512], f32)
            for ct in range(CT):
                nc.sync.dma_start(wv_sb[:, ct, :].bitcast(f32r),
                                  wv_d[128 * ct : 128 * (ct + 1), :].bitcast(f32r))
            wp_sb = wpp.tile([128, 4, 1024], f32)
            for ot in range(4):
                nc.sync.dma_start(wp_sb[:, ot, :].bitcast(f32r),
                                  wp_d[128 * ot : 128 * (ot + 1), :].bitcast(f32r))

            v_sb = vp.tile([128, NT, HPC, HD + 1], bf16)  # [k-part, k-tile, head, d | ones]
            nc.vector.memset(v_sb[:, :, :, HD], 1.0)

            ot_sb = otp.tile([128, PAIRS, N], f32)  # O^T rows: pair p = rows 128p..128p+127

            xT_r = xT_d.rearrange("(t p) n -> p t n", p=128)

            # ---- phase A: qkv projections (fp32r) ----
            # mt 0..3 = Q^T pairs, mt 4..7 = K^T pairs; emit K/Q for pair 0 first
            mt_order = [4, 0, 5, 1, 6, 2, 7, 3]
            for ach in range(N // ACH):
                xt = p16.tile([128, CT, ACH], f32, tag="big16")
                nc.sync.dma_start(xt.bitcast(f32r),
                                  xT_r[:, :, ACH * ach : ACH * (ach + 1)].bitcast(f32r))
                # V = x @ wv : out [n-tile, 512] -> v_sb (bf16 cast on evict)
                for sub in range(ACH // 128):
                    nt = (ACH // 128) * ach + sub
                    vps = psp.tile([128, 512], f32, tag="acc", bufs=3, name="vps")
                    for ct in range(CT):
                        mmr(vps, xt[:, ct, 128 * sub : 128 * (sub + 1)], wv_sb[:, ct, :],
                            start=(ct == 0), stop=(ct == CT - 1))
                    nc.vector.tensor_copy(
                        v_sb[:, nt, :, 0:HD],
                        vps.rearrange("p (h d) -> p h d", h=HPC),
                    )
                # qk^T = wqk^T @ x^T : out [m-tile, n-chunk] -> DRAM stage (bf16)
                for mt in mt_order:
                    qps = psp.tile([128, 512], f32, tag="acc", bufs=3, name="qps")
                    for ct in range(CT):
                        mmr(qps, wqk_sb[:, ct, 128 * mt : 128 * (mt + 1)], xt[:, ct, :],
                            start=(ct == 0), stop=(ct == CT - 1))
                    stg = miscp.tile([128, 512], bf16, tag="stg", bufs=3, name="stg")
                    nc.vector.tensor_copy(stg, qps)
                    nc.sync.dma_start(
                        qkT_d[128 * mt : 128 * (mt + 1), ACH * ach : ACH * (ach + 1)], stg
                    )

            # ---- phase B: attention per head-pair (bf16) ----
            for p in range(PAIRS):
                kt_sb = ktp.tile([128, N], bf16)  # K^T rows for both heads of the pair
                nc.sync.dma_start(kt_sb, qkT_d[512 + 128 * p : 512 + 128 * (p + 1), :])
                for qc in range(NQC):
                    qt_sb = qtp.tile([128, QCH], bf16)
                    nc.sync.dma_start(
                        qt_sb, qkT_d[128 * p : 128 * (p + 1), QCH * qc : QCH * (qc + 1)]
                    )
                    ptiles = [
                        p16.tile([128, KT, QCH], bf16, tag="big16", name=f"pt{hh}")
                        for hh in range(2)
                    ]
                    # S^T + exp: row-tiled pairs (head A rows 0:64, head B rows 64:128)
                    for b0, bn in S_BATCHES:
                        for hh in range(2):
                            sl = slice(64 * hh, 64 * (hh + 1))
                            sps = psp.tile([128, 4, QCH], f32, tag="sb4", bufs=1,
                                           name="sps")
                            for i in range(bn):
                                k = b0 + i
                                nc.tensor.matmul(
                                    sps[:, i, :], kt_sb[sl, 128 * k : 128 * (k + 1)],
                                    qt_sb[sl, :], start=True, stop=True)
                            nc.scalar.activation(
                                out=ptiles[hh][:, b0 : b0 + bn, :],
                                in_=sps[:, 0:bn, :],
                                func=EXP,
                                scale=float(SCALE),
                            )
                    # O' = [V | 1]^T @ P~^T, then normalize by fused row sums
                    for hh in range(2):
                        h = 2 * p + hh
                        ops = psp.tile([HD + 1, QCH], f32, tag="acc", bufs=3, name="ops")
                        for k in range(KT):
                            nc.tensor.matmul(ops, v_sb[:, k, h, :], ptiles[hh][:, k, :],
                                             start=(k == 0), stop=(k == KT - 1))
                        rcp = miscp.tile([1, QCH], f32, tag="rcp")
                        nc.vector.reciprocal(rcp, ops[HD : HD + 1, :])
                        # partition-broadcast via DRAM bounce (stride-0 partition
                        # APs are only legal on DRAM sources)
                        rcp_d = dp.tile([1, QCH], f32, tag="rcpd", bufs=4, name="rcpd")
                        nc.sync.dma_start(rcp_d, rcp)
                        bc = miscp.tile([64, QCH], f32, tag="bc")
                        rap = rcp_d[:]
                        nc.sync.dma_start(
                            bc,
                            bass.AP(tensor=rap.tensor, offset=rap.offset,
                                    ap=[[0, 64]] + list(rap.ap[1:])),
                        )
                        nc.vector.tensor_mul(
                            ot_sb[64 * hh : 64 * (hh + 1), p, QCH * qc : QCH * (qc + 1)].bitcast(f32r),
                            ops[0:HD, :],
                            bc,
                        )

            # ---- phase C: y = O @ wp (fp32r) ----
            for nt in range(NT):
                for yc in range(2):
                    yps = psp.tile([128, 512], f32, tag="acc", bufs=3, name="yps")
                    for ot in range(4):
                        mmr(yps, ot_sb[:, ot, 128 * nt : 128 * (nt + 1)],
                            wp_sb[:, ot, 512 * yc : 512 * (yc + 1)],
                            start=(ot == 0), stop=(ot == 3))
                    stg = miscp.tile([128, 512], f32, tag="ystg", bufs=3, name="ystg")
                    nc.vector.tensor_copy(stg, yps)
                    nc.sync.dma_start(
                        y_d[128 * nt : 128 * (nt + 1), 512 * yc : 512 * (yc + 1)], stg
                    )

    nc.compile()
    return nc


def get_nc():
    if "nc" not in _CACHE:
        _CACHE["nc"] = _build_nc()
    return _CACHE["nc"]


def make_in_maps(x, w_qkv, w_proj):
    in_maps = []
    for c in range(8):
        b, g = c // 2, c % 2
        in_maps.append({
            "xT": np.ascontiguousarray(x[b].T, dtype=np.float32),
            "wqk": np.ascontiguousarray(
                np.concatenate(
                    [w_qkv[:, 512 * g : 512 * (g + 1)],
                     w_qkv[:, 1024 + 512 * g : 1024 + 512 * (g + 1)]], axis=1
                ), dtype=np.float32),
            "wv": np.ascontiguousarray(
                w_qkv[:, 2048 + 512 * g : 2048 + 512 * (g + 1)], dtype=np.float32),
            "wp": np.ascontiguousarray(
                w_proj[512 * g : 512 * (g + 1), :], dtype=np.float32),
        })
    return in_maps


def kernel(x, w_qkv, w_proj, b_proj):
    from concourse.bass_utils import run_bass_kernel_spmd

    x = np.asarray(x, dtype=np.float32)
    w_qkv = np.asarray(w_qkv, dtype=np.float32)
    w_proj = np.asarray(w_proj, dtype=np.float32)
    b_proj = np.asarray(b_proj, dtype=np.float32)

    nc = get_nc()
    in_maps = make_in_maps(x, w_qkv, w_proj)
    res = run_bass_kernel_spmd(nc, in_maps, list(range(8))).results

    out = np.zeros((B, N, DIM), dtype=np.float32)
    for c in range(8):
        out[c // 2] += res[c]["y"]
    return out + b_proj
